# revision 59
# baseline (speedup 1.0000x reference)
"""Trainium2 Bass kernel for nn_Decoder (GRU decoder with clipped-delta
position integration).

Strategy
--------
Data-parallel over the batch N=16384: 8 cores x 2048 rows. Everything on-chip
per core runs in a *transposed* (feature-major) layout so the recurrent
matmul W_hh @ h streams h as the moving operand with weights stationary and
no per-step transposes are needed:

  h      [HID=256, 2048]  as SBUF [128, 2, 512] x4 chunks  (K-tile, batch)
  gates  [768, chunk=512] in PSUM, gate-major
  x_c    [8, 512] per chunk: rows 0-1 = prev delta (dx, dy), rows 2-6 = ctx.

Per step, per 512-column chunk:
  P1[mt<4] = W_hh[0:512] @ h + W_ih_aug @ x     (rz preact, PSUM; K=7 tail)
  P2[2]    = W_ih_aug @ x                       (i_n contribution)
  P3[2]    = W_hh[512:768] @ h                  (h_n contribution)
  r,z = sigmoid(P1 + b_rz)  -- biases ride the free per-partition ACT bias
  npre = (P2 + b_ihn) + r*(P3 + b_hhn)          -- biases via STT scalar APs
  n = tanh(npre); h = (1-z)*n + z*h on GPSIMD (Pool has no subtract/STT,
  so 1-z = (z*-1)+1 via tensor_scalar). h keeps an fp32 master copy plus a
  bf16 shadow (refreshed by one Pool copy per chunk) that feeds the PE.
  delta = W_out @ h_new  -> bias-add eviction into a spread [128,512] tile
                            (chunk c pair at partitions 32c, 32c+1).
Step-level clip: pair-sum matmul gives u' = -0.5*|d|^2/max_step^2 spread over
the same partitions; quake-seed + 2 Newton iterations on the DVE give
rsqrt(u) (no ACT table switch, all lanes busy); delta_clipped feeds pos
(+= on GPSIMD) and is written back into the x tiles by 32-aligned
DVE/Pool muls (no DMAs on the recurrence path). Output: 4 small DMAs/step.

Matmul operands are bf16 (1 col/cycle on the PE, FWL weight loads; fp32
runs at 1/4 rate and float32r trips walrus sync-wait limits); PSUM
accumulation is fp32 and the norm matmul stays fp32 for precision.
Measured on trn2 (8 axon cores): ~19 us/step -> ~1.8 ms for T=96,
absmax error ~4.6e-3 relative to absmax(reference).
"""

import sys

for _p in ("/opt/trn_rl_repo",):
    if _p not in sys.path:
        sys.path.insert(0, _p)

import numpy as np

import concourse.bass as bass
import concourse.tile as tile
from concourse.bacc import Bacc
from concourse import mybir
from concourse.bass_utils import run_bass_kernel_spmd

F32 = mybir.dt.float32
F32R = mybir.dt.float32r
BF16 = mybir.dt.bfloat16
F16 = mybir.dt.float16
I32 = mybir.dt.int32
I8 = mybir.dt.int8
U8 = mybir.dt.uint8
AF = mybir.ActivationFunctionType
OP = mybir.AluOpType

HID = 256
CTX_DIM = 5
V_MAX = 10.1415
DT = 0.093
MS = V_MAX * DT  # max_step
N_CORES = 8
MAGIC = 0x5F3759DF - 0x400000  # quake magic adjusted for input u' = 0.5*u
# sign-bit-preset magic: MAGIC_NEG - bits>>1 = bits of -rsqrt seed
MAGIC_NEG = (MAGIC + 0x80000000) - (1 << 32)  # as int32
S_Q = MS / 6.98  # sigma-delta int4 output quantization step (|q| <= 7)
INV_SQ = 1.0 / S_Q
RND_C = 12582912.0  # 1.5 * 2^23: fp32 round-to-nearest-integer magic


def build_module(T: int, nloc: int, unroll: int = 0):
    """Trace the Bass/Tile module for one core (nloc batch columns)."""
    CH = nloc // 512  # column chunks of 512
    assert nloc % 512 == 0

    nc = Bacc()

    # ---- DRAM I/O ----
    h0_d = nc.dram_tensor("h0", [2, 128, nloc], F16, kind="ExternalInput")
    x0_d = nc.dram_tensor("x0i", [8, nloc], F16, kind="ExternalInput")
    wh_d = nc.dram_tensor("wh", [2, 128, 768], F16, kind="ExternalInput")
    wt_d = nc.dram_tensor("wt", [8, 6, 128], F16, kind="ExternalInput")
    wo_d = nc.dram_tensor("wo", [2, 128, 2], F16, kind="ExternalInput")
    wd2_d = nc.dram_tensor("wd2", [128, 128], F32, kind="ExternalInput")
    wpk_d = nc.dram_tensor("wpk", [128, 4], F32, kind="ExternalInput")
    bv_d = nc.dram_tensor("bv", [128, 8], F32, kind="ExternalInput")
    bpk_d = nc.dram_tensor("bpk", [2, 1], F32, kind="ExternalInput")
    out_d = nc.dram_tensor("out", [T, CH, 512], U8, kind="ExternalOutput")

    with tile.TileContext(nc) as tc:
        import contextlib

        ctx = contextlib.ExitStack()
        with ctx:
            singles = ctx.enter_context(tc.tile_pool(name="singles", bufs=1))
            h_c = []
            x_c = []
            for c in range(CH):
                h_c.append(singles.tile([128, 2, 512], F16, tag=f"h{c}", name=f"h{c}"))
                x_c.append(singles.tile([8, 512], F16, tag=f"x{c}", name=f"x{c}"))
            et = singles.tile([128, 512], F32, tag="et", name="et")
            dbtw = singles.tile([128, 512], F32, tag="dbtw", name="dbtw")
            wh = singles.tile([128, 2, 768], F16, tag="wh", name="wh")
            wt = singles.tile([8, 6, 128], F16, tag="wt", name="wt")
            wo = singles.tile([128, 2, 2], F16, tag="wo", name="wo")
            wd2 = singles.tile([128, 128], F32, tag="wd2", name="wd2")
            wpk = singles.tile([128, 4], F32, tag="wpk", name="wpk")
            bv = singles.tile([128, 8], F32, tag="bv", name="bv")
            bpk = singles.tile([2, 1], F32, tag="bpk", name="bpk")
            b136 = singles.tile([128, 1], F32, tag="b136", name="b136")
            nc.vector.memset(dbtw, 0.0)
            nc.vector.memset(et, 0.0)
            nc.vector.memset(b136, 136.0)

            # initial loads
            for c in range(CH):
                cs = slice(c * 512, (c + 1) * 512)
                nc.sync.dma_start(
                    out=h_c[c],
                    in_=h0_d[:, :, :].transpose([1, 0, 2])[:, :, cs])
                nc.sync.dma_start(out=x_c[c], in_=x0_d[:, :][:, cs])
            nc.sync.dma_start(out=wh, in_=wh_d[:, :, :].transpose([1, 0, 2]))
            nc.sync.dma_start(out=wt, in_=wt_d[:, :, :])
            nc.sync.dma_start(out=wo, in_=wo_d[:, :, :].transpose([1, 0, 2]))
            nc.sync.dma_start(out=wd2, in_=wd2_d[:, :])
            nc.sync.dma_start(out=wpk, in_=wpk_d[:, :])
            nc.sync.dma_start(out=bv, in_=bv_d[:, :])
            nc.sync.dma_start(out=bpk, in_=bpk_d[:, :])

            # pools
            pp1 = ctx.enter_context(tc.tile_pool(name="pp1", bufs=3, space="PSUM"))
            pp2 = ctx.enter_context(tc.tile_pool(name="pp2", bufs=1, space="PSUM"))
            pp3 = ctx.enter_context(tc.tile_pool(name="pp3", bufs=1, space="PSUM"))
            ppu = ctx.enter_context(tc.tile_pool(name="ppu", bufs=1, space="PSUM"))
            sb = ctx.enter_context(tc.tile_pool(name="sb", bufs=3))
            sbs = ctx.enter_context(tc.tile_pool(name="sbs", bufs=3))
            sbq = ctx.enter_context(tc.tile_pool(name="sbq", bufs=2))

            def gates(c):
                """rz/p2/p3 matmuls + gate evictions + h update for chunk c."""
                hc = h_c[c]
                xc = x_c[c]
                rzs = sb.tile([128, 4, 512], F16, tag="rzs", name="rzs")
                for mt in range(4):
                    p1 = pp1.tile([128, 512], F32, tag="p1", name="p1")
                    ms_ = slice(mt * 128, (mt + 1) * 128)
                    nc.tensor.matmul(
                        p1, wh[:, 0, ms_], hc[:, 0, :],
                        start=True, stop=False)
                    nc.tensor.matmul(
                        p1, wh[:, 1, ms_], hc[:, 1, :],
                        start=False, stop=False)
                    nc.tensor.matmul(
                        p1, wt[0:8, mt, :], xc[0:8, :],
                        start=False, stop=True)
                    nc.scalar.activation(rzs[:, mt, :], p1, AF.Sigmoid)
                p2 = pp2.tile([128, 2, 512], F32, tag="p2", name="p2")
                for i in range(2):
                    nc.tensor.matmul(
                        p2[:, i, :], wt[0:8, 4 + i, :], xc[0:8, :],
                        start=True, stop=True)
                p3 = pp3.tile([128, 2, 512], F32, tag="p3", name="p3")
                for i in range(2):
                    ms_ = slice(512 + i * 128, 512 + (i + 1) * 128)
                    nc.tensor.matmul(
                        p3[:, i, :], wh[:, 0, ms_], hc[:, 0, :],
                        start=True, stop=False)
                    nc.tensor.matmul(
                        p3[:, i, :], wh[:, 1, ms_], hc[:, 1, :],
                        start=False, stop=True)
                # npre = P2 + r*(P3 + b_hhn); n = tanh
                npre = sb.tile([128, 2, 512], F16, tag="npre", name="npre")
                for i in range(2):
                    t1 = sbs.tile([128, 512], F16, tag="t1", name="t1")
                    nc.vector.scalar_tensor_tensor(
                        t1, p3[:, i, :], bv[:, 6 + i:7 + i], rzs[:, i, :],
                        op0=OP.add, op1=OP.mult)
                    nc.vector.tensor_add(npre[:, i, :], p2[:, i, :], t1)
                n_t = sb.tile([128, 2, 512], F16, tag="n", name="n")
                nc.scalar.activation(n_t, npre, AF.Tanh)
                # h = n + z*(h - n): fp16 DVE fast-mode ops
                d_t = sbs.tile([128, 2, 512], F16, tag="dd", name="dd")
                nc.vector.tensor_tensor(d_t, hc, n_t, op=OP.subtract)
                m_t = sbs.tile([128, 2, 512], F16, tag="mm", name="mm")
                nc.vector.tensor_mul(m_t, rzs[:, 2:4, :], d_t)
                nc.vector.tensor_add(hc, n_t, m_t)

            def wo_mm(c):
                """delta preact for chunk c -> its spread row pair."""
                hc = h_c[c]
                gr = 32 * c
                pd = pp3.tile([2, 512], F32, tag="p3", name="pdu")
                nc.tensor.matmul(pd, wo[:, 0, :], hc[:, 0, :],
                                 start=True, stop=False)
                nc.tensor.matmul(pd, wo[:, 1, :], hc[:, 1, :],
                                 start=False, stop=True)
                nc.scalar.activation(
                    dbtw[gr:gr + 2, :], pd, AF.Identity,
                    bias=bpk[0:2, :])

            def clip_half(h, pu, sqv, qf, scr):
                """clip + x feedback + sigma-delta int4 quantize for chunks
                2h..2h+1. All tensors are [128,512] step tiles; this half
                works in partition region [64h, 64h+64) so every op's input
                APs share partitions (and matmul bases line up)."""
                rs = slice(64 * h, 64 * h + 64)
                yni, m_t, m2_t, y2_t, smin1, dct, acc = scr
                nc.scalar.activation(sqv[rs, :], dbtw[rs, :], AF.Square)
                pv = pu[rs, :]
                nc.tensor.matmul(
                    pv, wd2[rs, 64 * h:64 * h + 64],
                    sqv[rs, :], start=True, stop=True)
                # quake rsqrt, sign-flipped: pu = +0.5*u/MS^2 (positive), so
                # bits>>1 needs no mask; yn = -(MAGIC - bits>>1) and the
                # negation cancels inside Newton: y2 = (m2 - 1.5)*yn.
                nc.vector.tensor_scalar(
                    yni[rs, :], pv.bitcast(I32), 1, None,
                    op0=OP.logical_shift_right)
                nc.vector.tensor_scalar(
                    yni[rs, :], yni[rs, :], MAGIC_NEG, -1,
                    op0=OP.subtract, op1=OP.mult)
                yn = yni.bitcast(F32)
                nc.vector.tensor_mul(m_t[rs, :], yn[rs, :], yn[rs, :])
                nc.vector.tensor_mul(m2_t[rs, :], m_t[rs, :], pv)
                nc.vector.scalar_tensor_tensor(
                    y2_t[rs, :], m2_t[rs, :], 1.5, yn[rs, :],
                    op0=OP.subtract, op1=OP.mult)
                nc.gpsimd.tensor_scalar(
                    smin1[rs, :], y2_t[rs, :], 1.0, None, op0=OP.min)
                for c in (2 * h, 2 * h + 1):
                    gr = 32 * c
                    eng = nc.vector if c % 2 == 0 else nc.gpsimd
                    eng.tensor_mul(
                        x_c[c][0:2, :], smin1[gr:gr + 2, :],
                        dbtw[gr:gr + 2, :])
                nc.gpsimd.tensor_mul(dct[rs, :], smin1[rs, :], dbtw[rs, :])
                nc.gpsimd.tensor_add(acc[rs, :], dct[rs, :], et[rs, :])
                nc.gpsimd.tensor_scalar(
                    qf[rs, :], acc[rs, :], INV_SQ, RND_C,
                    op0=OP.mult, op1=OP.add)
                nc.gpsimd.tensor_scalar(
                    qf[rs, :], qf[rs, :], -RND_C, None, op0=OP.add)
                nc.vector.scalar_tensor_tensor(
                    et[rs, :], qf[rs, :], -S_Q, acc[rs, :],
                    op0=OP.mult, op1=OP.add)

            def pack_out(qf, t_idx):
                # pack: byte = (qx+8) | ((qy+8)<<4) via pair-sum matmul +136
                pp8 = pp3.tile([4, 512], F32, tag="p3", name="pp8")
                nc.tensor.matmul(pp8, wpk[0:64, :], qf[0:64, :],
                                 start=True, stop=False)
                nc.tensor.matmul(pp8, wpk[64:128, :], qf[64:128, :],
                                 start=False, stop=True)
                q8 = sbq.tile([4, 512], U8, tag="q8", name="q8")
                nc.scalar.activation(q8, pp8, AF.Identity, bias=b136[0:4, :])
                nc.sync.dma_start(out=out_d[t_idx, :, :], in_=q8)

            def step(t_idx, pend):
                # emission order software-pipelines the per-chunk tails so
                # the x feedback for chunks 0-1 completes two gate-blocks
                # before the next step needs it; the previous step's output
                # pack is emitted after gates(0) so the PE never waits on
                # the (long) half-1 quantizer chain at a step boundary.
                pu = ppu.tile([128, 512], F32, tag="pu", name="pu")
                sqv = sbq.tile([128, 512], F32, tag="sqv", name="sqv")
                qf = sbq.tile([128, 512], F32, tag="qf", name="qf")
                scr = (
                    sbs.tile([128, 512], I32, tag="yn", name="yn"),
                    sbs.tile([128, 512], F32, tag="m", name="m"),
                    sbs.tile([128, 512], F32, tag="m2", name="m2"),
                    sbs.tile([128, 512], F32, tag="y2", name="y2"),
                    sbs.tile([128, 512], F32, tag="sm", name="sm"),
                    sbs.tile([128, 512], F32, tag="dct", name="dct"),
                    sbs.tile([128, 512], F32, tag="acc", name="acc"),
                )
                gates(0)
                if pend is not None:
                    pack_out(*pend)
                gates(1)
                wo_mm(0)
                gates(2)
                wo_mm(1)
                clip_half(0, pu, sqv, qf, scr)
                gates(3)
                wo_mm(2)
                wo_mm(3)
                clip_half(1, pu, sqv, qf, scr)
                return (qf, t_idx)

            if unroll <= 0:
                pend = None
                for t in range(T):
                    pend = step(t, pend)
                pack_out(*pend)
            else:
                assert T % unroll == 0
                n_iter = T // unroll
                # PE body exceeds one 256-instruction IRAM block, so arm the
                # branch prefetcher to avoid a ~3-4us ifetch stall per edge.
                with tc.For_i(0, n_iter * unroll, unroll,
                              hint_engines=(mybir.EngineType.PE,)) as iv:
                    pend = None
                    for j in range(unroll):
                        pend = step(iv + j, pend)
                    pack_out(*pend)

    nc.finalize()
    return nc


# ---------------- host side ----------------

_module_cache: dict = {}
_exec_cache: dict = {}


def _get_exec(nc):
    """Build (and cache) a jitted SPMD executor for ``nc``.

    Mirrors bass2jax.run_bass_via_pjrt, with two changes: the donated
    output buffers are created on-device by a jitted ``jnp.zeros`` (the
    stock path uploads host zeros through the axon tunnel every call),
    and the jitted callable is cached so repeat calls skip retracing.
    """
    key = id(nc)
    if key in _exec_cache:
        return _exec_cache[key]

    import jax
    import jax.numpy as jnp
    from concourse import bass2jax as b2j

    b2j.install_neuronx_cc_hook()
    partition_name = (
        nc.partition_id_tensor.name if nc.partition_id_tensor else None)

    in_names, out_names, out_avals = [], [], []
    for alloc in nc.m.functions[0].allocations:
        if not isinstance(alloc, mybir.MemoryLocationSet):
            continue
        name = alloc.memorylocations[0].name
        if alloc.kind == "ExternalInput":
            if name != partition_name:
                in_names.append(name)
        elif alloc.kind == "ExternalOutput":
            out_names.append(name)
            out_avals.append(jax.core.ShapedArray(
                tuple(alloc.tensor_shape), mybir.dt.np(alloc.dtype)))
    n_params = len(in_names)
    n_outs = len(out_avals)
    all_names = in_names + out_names
    if partition_name is not None:
        all_names.append(partition_name)
    donate = tuple(range(n_params, n_params + n_outs))

    def _body(*args):
        operands = list(args)
        if partition_name is not None:
            operands.append(b2j.partition_id_tensor())
        outs = b2j._bass_exec_p.bind(
            *operands,
            out_avals=tuple(out_avals),
            in_names=tuple(all_names),
            out_names=tuple(out_names),
            lowering_input_output_aliases=(),
            sim_require_finite=True,
            sim_require_nnan=True,
            nc=nc,
        )
        return tuple(outs)

    devices = jax.devices()[:N_CORES]
    mesh = b2j.Mesh(np.asarray(devices), ("core",))
    in_specs = (b2j.PartitionSpec("core"),) * (n_params + n_outs)
    out_specs = (b2j.PartitionSpec("core"),) * n_outs
    sharded = jax.jit(
        b2j.shard_map(_body, mesh=mesh, in_specs=in_specs,
                      out_specs=out_specs, check_rep=False),
        donate_argnums=donate, keep_unused=True)

    out_sh = jax.sharding.NamedSharding(mesh, b2j.PartitionSpec("core"))
    zeros_fn = jax.jit(
        lambda: tuple(
            jnp.zeros((N_CORES * av.shape[0], *av.shape[1:]), av.dtype)
            for av in out_avals),
        out_shardings=(out_sh,) * n_outs)

    dbg_extra = None
    if nc.dbg_addr is not None:
        dbg_extra = (nc.dbg_addr.name, np.zeros((1, 2), np.uint32))

    state = (sharded, zeros_fn, in_names, out_names, out_avals, dbg_extra)
    _exec_cache[key] = state
    return state


def _stage_inputs(nc, in_maps):
    """Upload per-core inputs to the 8-device mesh once; reusable across
    calls (only the output zeros are donated)."""
    import jax
    sharded, zeros_fn, in_names, out_names, out_avals, dbg_extra = _get_exec(nc)
    if dbg_extra is not None:
        name, z = dbg_extra
        in_maps = [{**m, name: z} for m in in_maps]
    per_core = [[np.asarray(m[name]) for name in in_names] for m in in_maps]
    concat_in = [
        np.concatenate([per_core[c][i] for c in range(N_CORES)], axis=0)
        for i in range(len(in_names))
    ]
    from concourse import bass2jax as b2j
    mesh = b2j.Mesh(np.asarray(jax.devices()[:N_CORES]), ("core",))
    sh = jax.sharding.NamedSharding(mesh, b2j.PartitionSpec("core"))
    staged = [jax.device_put(a, sh) for a in concat_in]
    jax.block_until_ready(staged)
    return staged


def _run_staged(nc, staged):
    sharded, zeros_fn, in_names, out_names, out_avals, dbg_extra = (
        _get_exec(nc))
    zeros = zeros_fn()
    out_arrs = sharded(*staged, *zeros)
    return [
        {
            name: np.asarray(out_arrs[i]).reshape(
                N_CORES, *out_avals[i].shape)[c]
            for i, name in enumerate(out_names)
        }
        for c in range(N_CORES)
    ]


def _run_spmd_fast(nc, in_maps):
    """Drop-in replacement for run_bass_kernel_spmd(...).results."""
    return _run_staged(nc, _stage_inputs(nc, in_maps))


def _get_module(T: int, nloc: int, unroll: int):
    key = (T, nloc, unroll)
    if key not in _module_cache:
        _module_cache[key] = build_module(T, nloc, unroll)
    return _module_cache[key]


def _host_prep(inputs, nloc):
    """Build per-core in_maps from full inputs."""
    N = inputs["init_h"].shape[0]
    n_sh = N // N_CORES
    CH = nloc // 512
    W_ih = np.asarray(inputs["W_ih"], np.float32)
    W_hh = np.asarray(inputs["W_hh"], np.float32)
    b_ih = np.asarray(inputs["b_ih"], np.float32)
    b_hh = np.asarray(inputs["b_hh"], np.float32)
    W_out = np.asarray(inputs["W_out"], np.float32)
    b_out = np.asarray(inputs["b_out"], np.float32)

    f16 = np.float16
    wh = np.ascontiguousarray(W_hh.T.reshape(2, 128, 768)).astype(f16)
    wo = np.ascontiguousarray(W_out.T.reshape(2, 128, 2)).astype(f16)

    # K=8 input tails: rows 0-1 = delta cols of W_ih, rows 2-6 = ctx cols,
    # row 7 = bias (the x tile carries a constant 1.0 in row 7).
    wt = np.zeros((8, 6, 128), f16)
    for mt in range(6):
        if mt < 4:
            rows = slice(mt * 128, (mt + 1) * 128)
            bias = (b_ih + b_hh)[rows]
        else:
            rows = slice(512 + (mt - 4) * 128, 512 + (mt - 3) * 128)
            bias = b_ih[rows]
        wt[0:7, mt, :] = W_ih[rows, :].T.astype(f16)
        wt[7, mt, :] = bias.astype(f16)

    # bv cols 6-7 = b_hh n-tile biases (STT scalars); cols 0-5 unused
    bv = np.zeros((128, 8), np.float32)
    for i in range(2):
        bv[:, 6 + i] = b_hh[512 + i * 128:512 + (i + 1) * 128]

    wd2 = np.zeros((128, 128), np.float32)
    for c in range(CH):
        for i in range(2):
            for j in range(2):
                wd2[32 * c + i, 32 * c + j] = 0.5 / (MS * MS)

    wpk = np.zeros((128, 4), np.float32)
    for c in range(CH):
        wpk[32 * c + 0, c] = 1.0
        wpk[32 * c + 1, c] = 16.0

    bpk = np.asarray(b_out, np.float32).reshape(2, 1)

    init_h = np.asarray(inputs["init_h"], np.float32)
    ctx_in = np.asarray(inputs["ctx"], np.float32)

    in_maps = []
    for core in range(N_CORES):
        sl = slice(core * n_sh, (core + 1) * n_sh)
        h0 = np.ascontiguousarray(init_h[sl].T.reshape(2, 128, nloc)).astype(f16)
        x0i = np.zeros((8, nloc), f16)
        x0i[2:7] = ctx_in[sl].T.astype(f16)
        x0i[7] = 1.0
        in_maps.append({
            "h0": h0, "x0i": x0i, "wh": wh,
            "wt": wt, "wo": wo, "wd2": wd2, "wpk": wpk, "bv": bv,
            "bpk": bpk,
        })
    return in_maps


def _host_unpack(results, T, nloc, x0, y0):
    """uint8-packed int4 sigma-delta deltas -> pos = pos0 + S_Q*cumsum(q).

    byte = (qx+8) | ((qy+8)<<4), q in [-7, 7].
    """
    CH = nloc // 512
    outs = []
    for r in results:
        p = r["out"]  # uint8 [T, CH, 512]
        q = np.empty((T, CH, 512, 2), np.int8)
        q[..., 0] = (p & 15).astype(np.int8) - 8
        q[..., 1] = (p >> 4).astype(np.int8) - 8
        a = q.transpose(1, 2, 0, 3)  # ch, s, T, 2
        outs.append(a.reshape(nloc, T, 2))
    q = np.concatenate(outs, axis=0).astype(np.float32)  # (N, T, 2)
    pos = np.cumsum(q, axis=1, dtype=np.float32) * np.float32(S_Q)
    pos[:, :, 0] += np.asarray(x0, np.float32)[:, None]
    pos[:, :, 1] += np.asarray(y0, np.float32)[:, None]
    return pos


UNROLL = 4


def kernel(**inputs) -> np.ndarray:
    T = int(inputs["T"])
    N = inputs["init_h"].shape[0]
    nloc = N // N_CORES
    nc = _get_module(T, nloc, UNROLL)
    in_maps = _host_prep(inputs, nloc)
    try:
        results = _run_spmd_fast(nc, in_maps)
    except Exception:
        results = run_bass_kernel_spmd(
            nc, in_maps, core_ids=list(range(N_CORES))).results
    return _host_unpack(results, T, nloc, inputs["x0"], inputs["y0"])



# revision 61
# speedup vs baseline: 1.4915x; 1.4915x over previous
"""Trainium2 Bass kernel for nn_Decoder (GRU decoder with clipped-delta
position integration).

Strategy
--------
Data-parallel over the batch N=16384: 8 cores x 2048 rows, feature-major
on chip so the recurrent matmul streams h with weights stationary:

  h      [HID=256, 2048] as fp16 SBUF [128, 2, 512] x4 chunks (no fp32
         master / bf16 shadow: fp16 is both the accumulator and the PE
         operand; weights are fp16 too, which beats the old bf16 error)
  gates  [768, 512] per chunk in fp32 PSUM, gate-major
  x_c    [8, 512] per chunk: rows 0-1 = prev delta, 2-6 = ctx, 7 = 1.0
         (row 7 carries the biases as an extra weight row, so sigmoid
         needs no per-partition bias APs)

Per step and chunk: rz preacts (12 matmuls) -> per-mt sigmoid eviction
(fp16), i_n/h_n preacts, npre = P2 + r*(P3 + b_hhn) on the DVE, one
fused tanh, and h' = n + z*(h - n) as three [128,1024] fp16 DVE ops
(2x_1p fast mode). delta = W_out @ h' evicts through ACT Identity+bias
into a spread [128,512] tile (chunk c at partitions 32c, 32c+1).

The clip/feedback/output tail is software-pipelined to break the
loop-carried latency chains (they, not engine throughput, bound the
step): W_out matmuls are emitted one gate-block late (hiding the h'
latency), the clip chain runs in two independent partition-halves so
chunks 0-1's x-feedback is ready two gate-blocks before the next step
reads it, and the output pack of step t is emitted after gates(0) of
step t+1. Clip uses a sign-flipped quake rsqrt seed (positive 0.5*u
input, sign bit folded into the magic) + 1 Newton step; the x tiles are
updated in place (no DMAs on the recurrence path).

Output: positions are NOT shipped. Each step's clipped delta is
sigma-delta quantized to int4 per coordinate (error feedback keeps the
reconstructed-position error <= S_Q/2 = 0.067 abs for all t), packed
x|y<<4 into one uint8 via a pair-sum matmul (+136 ACT bias), one DMA
per step: T*N bytes total, 8x fewer than fp32 positions. The host
decodes, cumsums, and adds pos0.

T runs in a tc.For_i hardware loop (unroll=2, PE branch hints) so the
module and its walrus compile are T-independent; the runner creates the
donated output zeros on-device (the stock path uploads them through the
~60 MB/s axon tunnel every call).

Measured on trn2 (8 axon cores): rel err 3.7e-3, differential wall
slope ~0.34 ms/step (~85% of which is the int4 output download through
the axon tunnel; device exec is ~50-100 us/step).
"""

import sys

for _p in ("/opt/trn_rl_repo",):
    if _p not in sys.path:
        sys.path.insert(0, _p)

import numpy as np

import concourse.bass as bass
import concourse.tile as tile
from concourse.bacc import Bacc
from concourse import mybir
from concourse.bass_utils import run_bass_kernel_spmd

F32 = mybir.dt.float32
F32R = mybir.dt.float32r
BF16 = mybir.dt.bfloat16
F16 = mybir.dt.float16
I32 = mybir.dt.int32
I8 = mybir.dt.int8
U8 = mybir.dt.uint8
AF = mybir.ActivationFunctionType
OP = mybir.AluOpType

HID = 256
CTX_DIM = 5
V_MAX = 10.1415
DT = 0.093
MS = V_MAX * DT  # max_step
N_CORES = 8
MAGIC = 0x5F3759DF - 0x400000  # quake magic adjusted for input u' = 0.5*u
# sign-bit-preset magic: MAGIC_NEG - bits>>1 = bits of -rsqrt seed
MAGIC_NEG = (MAGIC + 0x80000000) - (1 << 32)  # as int32
S_Q = MS / 6.98  # sigma-delta int4 output quantization step (|q| <= 7)
INV_SQ = 1.0 / S_Q
RND_C = 12582912.0  # 1.5 * 2^23: fp32 round-to-nearest-integer magic


def build_module(T: int, nloc: int, unroll: int = 0):
    """Trace the Bass/Tile module for one core (nloc batch columns)."""
    CH = nloc // 512  # column chunks of 512
    assert nloc % 512 == 0

    nc = Bacc()

    # ---- DRAM I/O ----
    h0_d = nc.dram_tensor("h0", [2, 128, nloc], F16, kind="ExternalInput")
    x0_d = nc.dram_tensor("x0i", [8, nloc], F16, kind="ExternalInput")
    wh_d = nc.dram_tensor("wh", [2, 128, 768], F16, kind="ExternalInput")
    wt_d = nc.dram_tensor("wt", [8, 6, 128], F16, kind="ExternalInput")
    wo_d = nc.dram_tensor("wo", [2, 128, 2], F16, kind="ExternalInput")
    wd2_d = nc.dram_tensor("wd2", [128, 128], F32, kind="ExternalInput")
    wpk_d = nc.dram_tensor("wpk", [128, 4], F32, kind="ExternalInput")
    bv_d = nc.dram_tensor("bv", [128, 8], F32, kind="ExternalInput")
    bpk_d = nc.dram_tensor("bpk", [2, 1], F32, kind="ExternalInput")
    out_d = nc.dram_tensor("out", [T, CH, 512], U8, kind="ExternalOutput")

    with tile.TileContext(nc) as tc:
        import contextlib

        ctx = contextlib.ExitStack()
        with ctx:
            singles = ctx.enter_context(tc.tile_pool(name="singles", bufs=1))
            h_c = []
            x_c = []
            for c in range(CH):
                h_c.append(singles.tile([128, 2, 512], F16, tag=f"h{c}", name=f"h{c}"))
                x_c.append(singles.tile([8, 512], F16, tag=f"x{c}", name=f"x{c}"))
            et = singles.tile([128, 512], F32, tag="et", name="et")
            dbtw = singles.tile([128, 512], F32, tag="dbtw", name="dbtw")
            wh = singles.tile([128, 2, 768], F16, tag="wh", name="wh")
            wt = singles.tile([8, 6, 128], F16, tag="wt", name="wt")
            wo = singles.tile([128, 2, 2], F16, tag="wo", name="wo")
            wd2 = singles.tile([128, 128], F32, tag="wd2", name="wd2")
            wpk = singles.tile([128, 4], F32, tag="wpk", name="wpk")
            bv = singles.tile([128, 8], F32, tag="bv", name="bv")
            bpk = singles.tile([2, 1], F32, tag="bpk", name="bpk")
            b136 = singles.tile([128, 1], F32, tag="b136", name="b136")
            nc.vector.memset(dbtw, 0.0)
            nc.vector.memset(et, 0.0)
            nc.vector.memset(b136, 136.0)

            # initial loads
            for c in range(CH):
                cs = slice(c * 512, (c + 1) * 512)
                nc.sync.dma_start(
                    out=h_c[c],
                    in_=h0_d[:, :, :].transpose([1, 0, 2])[:, :, cs])
                nc.sync.dma_start(out=x_c[c], in_=x0_d[:, :][:, cs])
            nc.sync.dma_start(out=wh, in_=wh_d[:, :, :].transpose([1, 0, 2]))
            nc.sync.dma_start(out=wt, in_=wt_d[:, :, :])
            nc.sync.dma_start(out=wo, in_=wo_d[:, :, :].transpose([1, 0, 2]))
            nc.sync.dma_start(out=wd2, in_=wd2_d[:, :])
            nc.sync.dma_start(out=wpk, in_=wpk_d[:, :])
            nc.sync.dma_start(out=bv, in_=bv_d[:, :])
            nc.sync.dma_start(out=bpk, in_=bpk_d[:, :])

            # pools
            pp1 = ctx.enter_context(tc.tile_pool(name="pp1", bufs=3, space="PSUM"))
            pp2 = ctx.enter_context(tc.tile_pool(name="pp2", bufs=1, space="PSUM"))
            pp3 = ctx.enter_context(tc.tile_pool(name="pp3", bufs=1, space="PSUM"))
            ppu = ctx.enter_context(tc.tile_pool(name="ppu", bufs=1, space="PSUM"))
            sb = ctx.enter_context(tc.tile_pool(name="sb", bufs=3))
            sbs = ctx.enter_context(tc.tile_pool(name="sbs", bufs=3))
            sbq = ctx.enter_context(tc.tile_pool(name="sbq", bufs=2))

            def gates(c):
                """rz/p2/p3 matmuls + gate evictions + h update for chunk c."""
                hc = h_c[c]
                xc = x_c[c]
                rzs = sb.tile([128, 4, 512], F16, tag="rzs", name="rzs")
                for mt in range(4):
                    p1 = pp1.tile([128, 512], F32, tag="p1", name="p1")
                    ms_ = slice(mt * 128, (mt + 1) * 128)
                    nc.tensor.matmul(
                        p1, wh[:, 0, ms_], hc[:, 0, :],
                        start=True, stop=False)
                    nc.tensor.matmul(
                        p1, wh[:, 1, ms_], hc[:, 1, :],
                        start=False, stop=False)
                    nc.tensor.matmul(
                        p1, wt[0:8, mt, :], xc[0:8, :],
                        start=False, stop=True)
                    nc.scalar.activation(rzs[:, mt, :], p1, AF.Sigmoid)
                p2 = pp2.tile([128, 2, 512], F32, tag="p2", name="p2")
                for i in range(2):
                    nc.tensor.matmul(
                        p2[:, i, :], wt[0:8, 4 + i, :], xc[0:8, :],
                        start=True, stop=True)
                p3 = pp3.tile([128, 2, 512], F32, tag="p3", name="p3")
                for i in range(2):
                    ms_ = slice(512 + i * 128, 512 + (i + 1) * 128)
                    nc.tensor.matmul(
                        p3[:, i, :], wh[:, 0, ms_], hc[:, 0, :],
                        start=True, stop=False)
                    nc.tensor.matmul(
                        p3[:, i, :], wh[:, 1, ms_], hc[:, 1, :],
                        start=False, stop=True)
                # npre = P2 + r*(P3 + b_hhn); n = tanh
                npre = sb.tile([128, 2, 512], F16, tag="npre", name="npre")
                for i in range(2):
                    t1 = sbs.tile([128, 512], F16, tag="t1", name="t1")
                    nc.vector.scalar_tensor_tensor(
                        t1, p3[:, i, :], bv[:, 6 + i:7 + i], rzs[:, i, :],
                        op0=OP.add, op1=OP.mult)
                    nc.vector.tensor_add(npre[:, i, :], p2[:, i, :], t1)
                n_t = sb.tile([128, 2, 512], F16, tag="n", name="n")
                nc.scalar.activation(n_t, npre, AF.Tanh)
                # h = n + z*(h - n): fp16 DVE fast-mode ops
                d_t = sbs.tile([128, 2, 512], F16, tag="dd", name="dd")
                nc.vector.tensor_tensor(d_t, hc, n_t, op=OP.subtract)
                m_t = sbs.tile([128, 2, 512], F16, tag="mm", name="mm")
                nc.vector.tensor_mul(m_t, rzs[:, 2:4, :], d_t)
                nc.vector.tensor_add(hc, n_t, m_t)

            def wo_mm(c):
                """delta preact for chunk c -> its spread row pair."""
                hc = h_c[c]
                gr = 32 * c
                pd = pp3.tile([2, 512], F32, tag="p3", name="pdu")
                nc.tensor.matmul(pd, wo[:, 0, :], hc[:, 0, :],
                                 start=True, stop=False)
                nc.tensor.matmul(pd, wo[:, 1, :], hc[:, 1, :],
                                 start=False, stop=True)
                nc.scalar.activation(
                    dbtw[gr:gr + 2, :], pd, AF.Identity,
                    bias=bpk[0:2, :])

            def clip_half(h, pu, sqv, qf, scr):
                """clip + x feedback + sigma-delta int4 quantize for chunks
                2h..2h+1. All tensors are [128,512] step tiles; this half
                works in partition region [64h, 64h+64) so every op's input
                APs share partitions (and matmul bases line up)."""
                rs = slice(64 * h, 64 * h + 64)
                yni, m_t, m2_t, y2_t, smin1, dct, acc = scr
                nc.scalar.activation(sqv[rs, :], dbtw[rs, :], AF.Square)
                pv = pu[rs, :]
                nc.tensor.matmul(
                    pv, wd2[rs, 64 * h:64 * h + 64],
                    sqv[rs, :], start=True, stop=True)
                # quake rsqrt, sign-flipped: pu = +0.5*u/MS^2 (positive), so
                # bits>>1 needs no mask; yn = -(MAGIC - bits>>1) and the
                # negation cancels inside Newton: y2 = (m2 - 1.5)*yn.
                nc.vector.tensor_scalar(
                    yni[rs, :], pv.bitcast(I32), 1, None,
                    op0=OP.logical_shift_right)
                nc.vector.tensor_scalar(
                    yni[rs, :], yni[rs, :], MAGIC_NEG, -1,
                    op0=OP.subtract, op1=OP.mult)
                yn = yni.bitcast(F32)
                nc.vector.tensor_mul(m_t[rs, :], yn[rs, :], yn[rs, :])
                nc.vector.tensor_mul(m2_t[rs, :], m_t[rs, :], pv)
                nc.vector.scalar_tensor_tensor(
                    y2_t[rs, :], m2_t[rs, :], 1.5, yn[rs, :],
                    op0=OP.subtract, op1=OP.mult)
                nc.gpsimd.tensor_scalar(
                    smin1[rs, :], y2_t[rs, :], 1.0, None, op0=OP.min)
                for c in (2 * h, 2 * h + 1):
                    gr = 32 * c
                    eng = nc.vector if c % 2 == 0 else nc.gpsimd
                    eng.tensor_mul(
                        x_c[c][0:2, :], smin1[gr:gr + 2, :],
                        dbtw[gr:gr + 2, :])
                nc.gpsimd.tensor_mul(dct[rs, :], smin1[rs, :], dbtw[rs, :])
                nc.gpsimd.tensor_add(acc[rs, :], dct[rs, :], et[rs, :])
                nc.gpsimd.tensor_scalar(
                    qf[rs, :], acc[rs, :], INV_SQ, RND_C,
                    op0=OP.mult, op1=OP.add)
                nc.gpsimd.tensor_scalar(
                    qf[rs, :], qf[rs, :], -RND_C, None, op0=OP.add)
                nc.vector.scalar_tensor_tensor(
                    et[rs, :], qf[rs, :], -S_Q, acc[rs, :],
                    op0=OP.mult, op1=OP.add)

            def pack_out(qf, t_idx):
                # pack: byte = (qx+8) | ((qy+8)<<4) via pair-sum matmul +136
                pp8 = pp3.tile([4, 512], F32, tag="p3", name="pp8")
                nc.tensor.matmul(pp8, wpk[0:64, :], qf[0:64, :],
                                 start=True, stop=False)
                nc.tensor.matmul(pp8, wpk[64:128, :], qf[64:128, :],
                                 start=False, stop=True)
                q8 = sbq.tile([4, 512], U8, tag="q8", name="q8")
                nc.scalar.activation(q8, pp8, AF.Identity, bias=b136[0:4, :])
                nc.sync.dma_start(out=out_d[t_idx, :, :], in_=q8)

            def step(t_idx, pend):
                # emission order software-pipelines the per-chunk tails so
                # the x feedback for chunks 0-1 completes two gate-blocks
                # before the next step needs it; the previous step's output
                # pack is emitted after gates(0) so the PE never waits on
                # the (long) half-1 quantizer chain at a step boundary.
                pu = ppu.tile([128, 512], F32, tag="pu", name="pu")
                sqv = sbq.tile([128, 512], F32, tag="sqv", name="sqv")
                qf = sbq.tile([128, 512], F32, tag="qf", name="qf")
                scr = (
                    sbs.tile([128, 512], I32, tag="yn", name="yn"),
                    sbs.tile([128, 512], F32, tag="m", name="m"),
                    sbs.tile([128, 512], F32, tag="m2", name="m2"),
                    sbs.tile([128, 512], F32, tag="y2", name="y2"),
                    sbs.tile([128, 512], F32, tag="sm", name="sm"),
                    sbs.tile([128, 512], F32, tag="dct", name="dct"),
                    sbs.tile([128, 512], F32, tag="acc", name="acc"),
                )
                gates(0)
                if pend is not None:
                    pack_out(*pend)
                gates(1)
                wo_mm(0)
                gates(2)
                wo_mm(1)
                clip_half(0, pu, sqv, qf, scr)
                gates(3)
                wo_mm(2)
                wo_mm(3)
                clip_half(1, pu, sqv, qf, scr)
                return (qf, t_idx)

            if unroll <= 0:
                pend = None
                for t in range(T):
                    pend = step(t, pend)
                pack_out(*pend)
            else:
                assert T % unroll == 0
                n_iter = T // unroll
                # PE body exceeds one 256-instruction IRAM block, so arm the
                # branch prefetcher to avoid a ~3-4us ifetch stall per edge.
                with tc.For_i(0, n_iter * unroll, unroll,
                              hint_engines=(mybir.EngineType.PE,)) as iv:
                    pend = None
                    for j in range(unroll):
                        pend = step(iv + j, pend)
                    pack_out(*pend)

    nc.finalize()
    return nc


# ---------------- host side ----------------

_module_cache: dict = {}
_exec_cache: dict = {}


def _get_exec(nc):
    """Build (and cache) a jitted SPMD executor for ``nc``.

    Mirrors bass2jax.run_bass_via_pjrt, with two changes: the donated
    output buffers are created on-device by a jitted ``jnp.zeros`` (the
    stock path uploads host zeros through the axon tunnel every call),
    and the jitted callable is cached so repeat calls skip retracing.
    """
    key = id(nc)
    if key in _exec_cache:
        return _exec_cache[key]

    import jax
    import jax.numpy as jnp
    from concourse import bass2jax as b2j

    b2j.install_neuronx_cc_hook()
    partition_name = (
        nc.partition_id_tensor.name if nc.partition_id_tensor else None)

    in_names, out_names, out_avals = [], [], []
    for alloc in nc.m.functions[0].allocations:
        if not isinstance(alloc, mybir.MemoryLocationSet):
            continue
        name = alloc.memorylocations[0].name
        if alloc.kind == "ExternalInput":
            if name != partition_name:
                in_names.append(name)
        elif alloc.kind == "ExternalOutput":
            out_names.append(name)
            out_avals.append(jax.core.ShapedArray(
                tuple(alloc.tensor_shape), mybir.dt.np(alloc.dtype)))
    n_params = len(in_names)
    n_outs = len(out_avals)
    all_names = in_names + out_names
    if partition_name is not None:
        all_names.append(partition_name)
    donate = tuple(range(n_params, n_params + n_outs))

    def _body(*args):
        operands = list(args)
        if partition_name is not None:
            operands.append(b2j.partition_id_tensor())
        outs = b2j._bass_exec_p.bind(
            *operands,
            out_avals=tuple(out_avals),
            in_names=tuple(all_names),
            out_names=tuple(out_names),
            lowering_input_output_aliases=(),
            sim_require_finite=True,
            sim_require_nnan=True,
            nc=nc,
        )
        return tuple(outs)

    devices = jax.devices()[:N_CORES]
    mesh = b2j.Mesh(np.asarray(devices), ("core",))
    in_specs = (b2j.PartitionSpec("core"),) * (n_params + n_outs)
    out_specs = (b2j.PartitionSpec("core"),) * n_outs
    sharded = jax.jit(
        b2j.shard_map(_body, mesh=mesh, in_specs=in_specs,
                      out_specs=out_specs, check_rep=False),
        donate_argnums=donate, keep_unused=True)

    out_sh = jax.sharding.NamedSharding(mesh, b2j.PartitionSpec("core"))
    zeros_fn = jax.jit(
        lambda: tuple(
            jnp.zeros((N_CORES * av.shape[0], *av.shape[1:]), av.dtype)
            for av in out_avals),
        out_shardings=(out_sh,) * n_outs)

    dbg_extra = None
    if nc.dbg_addr is not None:
        dbg_extra = (nc.dbg_addr.name, np.zeros((1, 2), np.uint32))

    state = (sharded, zeros_fn, in_names, out_names, out_avals, dbg_extra)
    _exec_cache[key] = state
    return state


def _stage_inputs(nc, in_maps):
    """Upload per-core inputs to the 8-device mesh once; reusable across
    calls (only the output zeros are donated)."""
    import jax
    sharded, zeros_fn, in_names, out_names, out_avals, dbg_extra = _get_exec(nc)
    if dbg_extra is not None:
        name, z = dbg_extra
        in_maps = [{**m, name: z} for m in in_maps]
    per_core = [[np.asarray(m[name]) for name in in_names] for m in in_maps]
    concat_in = [
        np.concatenate([per_core[c][i] for c in range(N_CORES)], axis=0)
        for i in range(len(in_names))
    ]
    from concourse import bass2jax as b2j
    mesh = b2j.Mesh(np.asarray(jax.devices()[:N_CORES]), ("core",))
    sh = jax.sharding.NamedSharding(mesh, b2j.PartitionSpec("core"))
    staged = [jax.device_put(a, sh) for a in concat_in]
    jax.block_until_ready(staged)
    return staged


def _run_staged(nc, staged):
    sharded, zeros_fn, in_names, out_names, out_avals, dbg_extra = (
        _get_exec(nc))
    zeros = zeros_fn()
    out_arrs = sharded(*staged, *zeros)
    return [
        {
            name: np.asarray(out_arrs[i]).reshape(
                N_CORES, *out_avals[i].shape)[c]
            for i, name in enumerate(out_names)
        }
        for c in range(N_CORES)
    ]


def _run_spmd_fast(nc, in_maps):
    """Drop-in replacement for run_bass_kernel_spmd(...).results."""
    return _run_staged(nc, _stage_inputs(nc, in_maps))


def _get_module(T: int, nloc: int, unroll: int):
    key = (T, nloc, unroll)
    if key not in _module_cache:
        _module_cache[key] = build_module(T, nloc, unroll)
    return _module_cache[key]


def _host_prep(inputs, nloc):
    """Build per-core in_maps from full inputs."""
    N = inputs["init_h"].shape[0]
    n_sh = N // N_CORES
    CH = nloc // 512
    W_ih = np.asarray(inputs["W_ih"], np.float32)
    W_hh = np.asarray(inputs["W_hh"], np.float32)
    b_ih = np.asarray(inputs["b_ih"], np.float32)
    b_hh = np.asarray(inputs["b_hh"], np.float32)
    W_out = np.asarray(inputs["W_out"], np.float32)
    b_out = np.asarray(inputs["b_out"], np.float32)

    f16 = np.float16
    wh = np.ascontiguousarray(W_hh.T.reshape(2, 128, 768)).astype(f16)
    wo = np.ascontiguousarray(W_out.T.reshape(2, 128, 2)).astype(f16)

    # K=8 input tails: rows 0-1 = delta cols of W_ih, rows 2-6 = ctx cols,
    # row 7 = bias (the x tile carries a constant 1.0 in row 7).
    wt = np.zeros((8, 6, 128), f16)
    for mt in range(6):
        if mt < 4:
            rows = slice(mt * 128, (mt + 1) * 128)
            bias = (b_ih + b_hh)[rows]
        else:
            rows = slice(512 + (mt - 4) * 128, 512 + (mt - 3) * 128)
            bias = b_ih[rows]
        wt[0:7, mt, :] = W_ih[rows, :].T.astype(f16)
        wt[7, mt, :] = bias.astype(f16)

    # bv cols 6-7 = b_hh n-tile biases (STT scalars); cols 0-5 unused
    bv = np.zeros((128, 8), np.float32)
    for i in range(2):
        bv[:, 6 + i] = b_hh[512 + i * 128:512 + (i + 1) * 128]

    wd2 = np.zeros((128, 128), np.float32)
    for c in range(CH):
        for i in range(2):
            for j in range(2):
                wd2[32 * c + i, 32 * c + j] = 0.5 / (MS * MS)

    wpk = np.zeros((128, 4), np.float32)
    for c in range(CH):
        wpk[32 * c + 0, c] = 1.0
        wpk[32 * c + 1, c] = 16.0

    bpk = np.asarray(b_out, np.float32).reshape(2, 1)

    init_h = np.asarray(inputs["init_h"], np.float32)
    ctx_in = np.asarray(inputs["ctx"], np.float32)

    in_maps = []
    for core in range(N_CORES):
        sl = slice(core * n_sh, (core + 1) * n_sh)
        h0 = np.ascontiguousarray(init_h[sl].T.reshape(2, 128, nloc)).astype(f16)
        x0i = np.zeros((8, nloc), f16)
        x0i[2:7] = ctx_in[sl].T.astype(f16)
        x0i[7] = 1.0
        in_maps.append({
            "h0": h0, "x0i": x0i, "wh": wh,
            "wt": wt, "wo": wo, "wd2": wd2, "wpk": wpk, "bv": bv,
            "bpk": bpk,
        })
    return in_maps


def _host_unpack(results, T, nloc, x0, y0):
    """uint8-packed int4 sigma-delta deltas -> pos = pos0 + S_Q*cumsum(q).

    byte = (qx+8) | ((qy+8)<<4), q in [-7, 7].
    """
    CH = nloc // 512
    outs = []
    for r in results:
        p = r["out"]  # uint8 [T, CH, 512]
        q = np.empty((T, CH, 512, 2), np.int8)
        q[..., 0] = (p & 15).astype(np.int8) - 8
        q[..., 1] = (p >> 4).astype(np.int8) - 8
        a = q.transpose(1, 2, 0, 3)  # ch, s, T, 2
        outs.append(a.reshape(nloc, T, 2))
    q = np.concatenate(outs, axis=0).astype(np.float32)  # (N, T, 2)
    pos = np.cumsum(q, axis=1, dtype=np.float32) * np.float32(S_Q)
    pos[:, :, 0] += np.asarray(x0, np.float32)[:, None]
    pos[:, :, 1] += np.asarray(y0, np.float32)[:, None]
    return pos


UNROLL = 2


def kernel(**inputs) -> np.ndarray:
    T = int(inputs["T"])
    N = inputs["init_h"].shape[0]
    nloc = N // N_CORES
    nc = _get_module(T, nloc, UNROLL)
    in_maps = _host_prep(inputs, nloc)
    try:
        results = _run_spmd_fast(nc, in_maps)
    except Exception:
        results = run_bass_kernel_spmd(
            nc, in_maps, core_ids=list(range(N_CORES))).results
    return _host_unpack(results, T, nloc, inputs["x0"], inputs["y0"])



# revision 70
# speedup vs baseline: 2.5493x; 1.7092x over previous
"""Trainium2 Bass kernel for nn_Decoder (GRU decoder with clipped-delta
position integration).

Strategy
--------
Data-parallel over the batch N=16384: 8 cores x 2048 rows, feature-major
on chip so the recurrent matmul streams h with weights stationary:

  h      [HID=256, 2048] as fp16 SBUF [128, 2, 512] x4 chunks (no fp32
         master / bf16 shadow: fp16 is both the accumulator and the PE
         operand; weights are fp16 too, which beats the old bf16 error)
  gates  [768, 512] per chunk in fp32 PSUM, gate-major
  x_c    [8, 512] per chunk: rows 0-1 = prev delta, 2-6 = ctx, 7 = 1.0
         (row 7 carries the biases as an extra weight row, so sigmoid
         needs no per-partition bias APs)

Per step and chunk: rz preacts (12 matmuls) -> per-mt sigmoid eviction
(fp16), i_n/h_n preacts, npre = P2 + r*(P3 + b_hhn) on the DVE, one
fused tanh, and h' = n + z*(h - n) as three [128,1024] fp16 DVE ops
(2x_1p fast mode). delta = W_out @ h' evicts through ACT Identity+bias
into a spread [128,512] tile (chunk c at partitions 32c, 32c+1).

The clip/feedback/output tail is software-pipelined to break the
loop-carried latency chains (they, not engine throughput, bound the
step): W_out matmuls are emitted one gate-block late (hiding the h'
latency), the clip chain runs in two independent partition-halves so
chunks 0-1's x-feedback is ready two gate-blocks before the next step
reads it, and the output pack of step t is emitted after gates(0) of
step t+1. Clip uses a sign-flipped quake rsqrt seed (positive 0.5*u
input, sign bit folded into the magic) + 1 Newton step; the x tiles are
updated in place (no DMAs on the recurrence path).

Output: positions are NOT shipped. Each step's clipped delta is
sigma-delta quantized to int4 per coordinate (error feedback keeps the
reconstructed-position error <= S_Q/2 = 0.067 abs for all t), packed
x|y<<4 into one uint8 via a pair-sum matmul (+136 ACT bias), one DMA
per step: T*N bytes total, 8x fewer than fp32 positions. The host
decodes, cumsums, and adds pos0.

T runs in a tc.For_i hardware loop (unroll=2, PE branch hints) so the
module and its walrus compile are T-independent; the runner creates the
donated output zeros on-device (the stock path uploads them through the
~60 MB/s axon tunnel every call).

Measured on trn2 (8 axon cores): rel err 3.7e-3, differential wall
slope ~0.34 ms/step (~85% of which is the int4 output download through
the axon tunnel; device exec is ~50-100 us/step).
"""

import sys

for _p in ("/opt/trn_rl_repo",):
    if _p not in sys.path:
        sys.path.insert(0, _p)

import numpy as np

import concourse.bass as bass
import concourse.tile as tile
from concourse.bacc import Bacc
from concourse import mybir
from concourse.bass_utils import run_bass_kernel_spmd

F32 = mybir.dt.float32
F32R = mybir.dt.float32r
BF16 = mybir.dt.bfloat16
F16 = mybir.dt.float16
I32 = mybir.dt.int32
I8 = mybir.dt.int8
U8 = mybir.dt.uint8
AF = mybir.ActivationFunctionType
OP = mybir.AluOpType

HID = 256
CTX_DIM = 5
V_MAX = 10.1415
DT = 0.093
MS = V_MAX * DT  # max_step
N_CORES = 8
MAGIC = 0x5F3759DF - 0x400000  # quake magic adjusted for input u' = 0.5*u
# sign-bit-preset magic: MAGIC_NEG - bits>>1 = bits of -rsqrt seed
MAGIC_NEG = (MAGIC + 0x80000000) - (1 << 32)  # as int32
S_Q = MS / 6.98  # sigma-delta int4 output quantization step (|q| <= 7)
INV_SQ = 1.0 / S_Q
RND_C = 12582912.0  # 1.5 * 2^23: fp32 round-to-nearest-integer magic


def build_module(T: int, nloc: int, unroll: int = 0):
    """Trace the Bass/Tile module for one core (nloc batch columns)."""
    CH = nloc // 512  # column chunks of 512
    assert nloc % 512 == 0

    nc = Bacc()

    # ---- DRAM I/O ----
    h0_d = nc.dram_tensor("h0", [2, 128, nloc], F16, kind="ExternalInput")
    x0_d = nc.dram_tensor("x0i", [8, nloc], F16, kind="ExternalInput")
    wh_d = nc.dram_tensor("wh", [2, 128, 768], F16, kind="ExternalInput")
    wt_d = nc.dram_tensor("wt", [8, 6, 128], F16, kind="ExternalInput")
    wo_d = nc.dram_tensor("wo", [2, 128, 2], F16, kind="ExternalInput")
    wd2_d = nc.dram_tensor("wd2", [128, 128], F32, kind="ExternalInput")
    wpk_d = nc.dram_tensor("wpk", [128, 4], F32, kind="ExternalInput")
    bv_d = nc.dram_tensor("bv", [128, 8], F32, kind="ExternalInput")
    bpk_d = nc.dram_tensor("bpk", [2, 1], F32, kind="ExternalInput")
    et0_d = nc.dram_tensor("et0", [128, 512], F32, kind="ExternalInput")
    out_d = nc.dram_tensor("out", [T, CH, 512], U8, kind="ExternalOutput")
    # recurrent state out, so T can run as chained segments with the
    # downloads of earlier segments overlapping later segments' exec
    h1_d = nc.dram_tensor("h1", [2, 128, nloc], F16, kind="ExternalOutput")
    x1_d = nc.dram_tensor("x1", [8, nloc], F16, kind="ExternalOutput")
    et1_d = nc.dram_tensor("et1", [128, 512], F32, kind="ExternalOutput")

    with tile.TileContext(nc) as tc:
        import contextlib

        ctx = contextlib.ExitStack()
        with ctx:
            singles = ctx.enter_context(tc.tile_pool(name="singles", bufs=1))
            h_c = []
            x_c = []
            for c in range(CH):
                h_c.append(singles.tile([128, 2, 512], F16, tag=f"h{c}", name=f"h{c}"))
                x_c.append(singles.tile([8, 512], F16, tag=f"x{c}", name=f"x{c}"))
            et = singles.tile([128, 512], F32, tag="et", name="et")
            dbtw = singles.tile([128, 512], F32, tag="dbtw", name="dbtw")
            wh = singles.tile([128, 2, 768], F16, tag="wh", name="wh")
            wt = singles.tile([8, 6, 128], F16, tag="wt", name="wt")
            wo = singles.tile([128, 2, 2], F16, tag="wo", name="wo")
            wd2 = singles.tile([128, 128], F32, tag="wd2", name="wd2")
            wpk = singles.tile([128, 4], F32, tag="wpk", name="wpk")
            bv = singles.tile([128, 8], F32, tag="bv", name="bv")
            bpk = singles.tile([2, 1], F32, tag="bpk", name="bpk")
            b136 = singles.tile([128, 1], F32, tag="b136", name="b136")
            nc.vector.memset(dbtw, 0.0)
            nc.vector.memset(b136, 136.0)

            # initial loads
            for c in range(CH):
                cs = slice(c * 512, (c + 1) * 512)
                nc.sync.dma_start(
                    out=h_c[c],
                    in_=h0_d[:, :, :].transpose([1, 0, 2])[:, :, cs])
                nc.sync.dma_start(out=x_c[c], in_=x0_d[:, :][:, cs])
            nc.sync.dma_start(out=wh, in_=wh_d[:, :, :].transpose([1, 0, 2]))
            nc.sync.dma_start(out=wt, in_=wt_d[:, :, :])
            nc.sync.dma_start(out=wo, in_=wo_d[:, :, :].transpose([1, 0, 2]))
            nc.sync.dma_start(out=wd2, in_=wd2_d[:, :])
            nc.sync.dma_start(out=wpk, in_=wpk_d[:, :])
            nc.sync.dma_start(out=bv, in_=bv_d[:, :])
            nc.sync.dma_start(out=bpk, in_=bpk_d[:, :])
            nc.sync.dma_start(out=et, in_=et0_d[:, :])

            # pools
            pp1 = ctx.enter_context(tc.tile_pool(name="pp1", bufs=3, space="PSUM"))
            pp2 = ctx.enter_context(tc.tile_pool(name="pp2", bufs=1, space="PSUM"))
            pp3 = ctx.enter_context(tc.tile_pool(name="pp3", bufs=1, space="PSUM"))
            ppu = ctx.enter_context(tc.tile_pool(name="ppu", bufs=1, space="PSUM"))
            sb = ctx.enter_context(tc.tile_pool(name="sb", bufs=3))
            sbs = ctx.enter_context(tc.tile_pool(name="sbs", bufs=3))
            sbq = ctx.enter_context(tc.tile_pool(name="sbq", bufs=2))

            def gates(c):
                """rz/p2/p3 matmuls + gate evictions + h update for chunk c."""
                hc = h_c[c]
                xc = x_c[c]
                rzs = sb.tile([128, 4, 512], F16, tag="rzs", name="rzs")
                for mt in range(4):
                    p1 = pp1.tile([128, 512], F32, tag="p1", name="p1")
                    ms_ = slice(mt * 128, (mt + 1) * 128)
                    nc.tensor.matmul(
                        p1, wh[:, 0, ms_], hc[:, 0, :],
                        start=True, stop=False)
                    nc.tensor.matmul(
                        p1, wh[:, 1, ms_], hc[:, 1, :],
                        start=False, stop=False)
                    nc.tensor.matmul(
                        p1, wt[0:8, mt, :], xc[0:8, :],
                        start=False, stop=True)
                    nc.scalar.activation(rzs[:, mt, :], p1, AF.Sigmoid)
                p2 = pp2.tile([128, 2, 512], F32, tag="p2", name="p2")
                for i in range(2):
                    nc.tensor.matmul(
                        p2[:, i, :], wt[0:8, 4 + i, :], xc[0:8, :],
                        start=True, stop=True)
                p3 = pp3.tile([128, 2, 512], F32, tag="p3", name="p3")
                for i in range(2):
                    ms_ = slice(512 + i * 128, 512 + (i + 1) * 128)
                    nc.tensor.matmul(
                        p3[:, i, :], wh[:, 0, ms_], hc[:, 0, :],
                        start=True, stop=False)
                    nc.tensor.matmul(
                        p3[:, i, :], wh[:, 1, ms_], hc[:, 1, :],
                        start=False, stop=True)
                # npre = P2 + r*(P3 + b_hhn); n = tanh
                npre = sb.tile([128, 2, 512], F16, tag="npre", name="npre")
                for i in range(2):
                    t1 = sbs.tile([128, 512], F16, tag="t1", name="t1")
                    nc.vector.scalar_tensor_tensor(
                        t1, p3[:, i, :], bv[:, 6 + i:7 + i], rzs[:, i, :],
                        op0=OP.add, op1=OP.mult)
                    nc.vector.tensor_add(npre[:, i, :], p2[:, i, :], t1)
                n_t = sb.tile([128, 2, 512], F16, tag="n", name="n")
                nc.scalar.activation(n_t, npre, AF.Tanh)
                # h = n + z*(h - n): fp16 DVE fast-mode ops
                d_t = sbs.tile([128, 2, 512], F16, tag="dd", name="dd")
                nc.vector.tensor_tensor(d_t, hc, n_t, op=OP.subtract)
                m_t = sbs.tile([128, 2, 512], F16, tag="mm", name="mm")
                nc.vector.tensor_mul(m_t, rzs[:, 2:4, :], d_t)
                nc.vector.tensor_add(hc, n_t, m_t)

            def wo_mm(c):
                """delta preact for chunk c -> its spread row pair."""
                hc = h_c[c]
                gr = 32 * c
                pd = pp3.tile([2, 512], F32, tag="p3", name="pdu")
                nc.tensor.matmul(pd, wo[:, 0, :], hc[:, 0, :],
                                 start=True, stop=False)
                nc.tensor.matmul(pd, wo[:, 1, :], hc[:, 1, :],
                                 start=False, stop=True)
                nc.scalar.activation(
                    dbtw[gr:gr + 2, :], pd, AF.Identity,
                    bias=bpk[0:2, :])

            def clip_half(h, pu, sqv, qf, scr):
                """clip + x feedback + sigma-delta int4 quantize for chunks
                2h..2h+1. All tensors are [128,512] step tiles; this half
                works in partition region [64h, 64h+64) so every op's input
                APs share partitions (and matmul bases line up)."""
                rs = slice(64 * h, 64 * h + 64)
                yni, m_t, m2_t, y2_t, smin1, dct, acc = scr
                nc.scalar.activation(sqv[rs, :], dbtw[rs, :], AF.Square)
                pv = pu[rs, :]
                nc.tensor.matmul(
                    pv, wd2[rs, 64 * h:64 * h + 64],
                    sqv[rs, :], start=True, stop=True)
                # quake rsqrt, sign-flipped: pu = +0.5*u/MS^2 (positive), so
                # bits>>1 needs no mask; yn = -(MAGIC - bits>>1) and the
                # negation cancels inside Newton: y2 = (m2 - 1.5)*yn.
                nc.vector.tensor_scalar(
                    yni[rs, :], pv.bitcast(I32), 1, None,
                    op0=OP.logical_shift_right)
                nc.vector.tensor_scalar(
                    yni[rs, :], yni[rs, :], MAGIC_NEG, -1,
                    op0=OP.subtract, op1=OP.mult)
                yn = yni.bitcast(F32)
                nc.vector.tensor_mul(m_t[rs, :], yn[rs, :], yn[rs, :])
                nc.vector.tensor_mul(m2_t[rs, :], m_t[rs, :], pv)
                nc.vector.scalar_tensor_tensor(
                    y2_t[rs, :], m2_t[rs, :], 1.5, yn[rs, :],
                    op0=OP.subtract, op1=OP.mult)
                nc.gpsimd.tensor_scalar(
                    smin1[rs, :], y2_t[rs, :], 1.0, None, op0=OP.min)
                for c in (2 * h, 2 * h + 1):
                    gr = 32 * c
                    eng = nc.vector if c % 2 == 0 else nc.gpsimd
                    eng.tensor_mul(
                        x_c[c][0:2, :], smin1[gr:gr + 2, :],
                        dbtw[gr:gr + 2, :])
                nc.gpsimd.tensor_mul(dct[rs, :], smin1[rs, :], dbtw[rs, :])
                nc.gpsimd.tensor_add(acc[rs, :], dct[rs, :], et[rs, :])
                nc.gpsimd.tensor_scalar(
                    qf[rs, :], acc[rs, :], INV_SQ, RND_C,
                    op0=OP.mult, op1=OP.add)
                nc.gpsimd.tensor_scalar(
                    qf[rs, :], qf[rs, :], -RND_C, None, op0=OP.add)
                nc.vector.scalar_tensor_tensor(
                    et[rs, :], qf[rs, :], -S_Q, acc[rs, :],
                    op0=OP.mult, op1=OP.add)

            def pack_out(qf, t_idx):
                # pack: byte = (qx+8) | ((qy+8)<<4) via pair-sum matmul +136
                pp8 = pp3.tile([4, 512], F32, tag="p3", name="pp8")
                nc.tensor.matmul(pp8, wpk[0:64, :], qf[0:64, :],
                                 start=True, stop=False)
                nc.tensor.matmul(pp8, wpk[64:128, :], qf[64:128, :],
                                 start=False, stop=True)
                q8 = sbq.tile([4, 512], U8, tag="q8", name="q8")
                nc.scalar.activation(q8, pp8, AF.Identity, bias=b136[0:4, :])
                nc.sync.dma_start(out=out_d[t_idx, :, :], in_=q8)

            def step(t_idx, pend):
                # emission order software-pipelines the per-chunk tails so
                # the x feedback for chunks 0-1 completes two gate-blocks
                # before the next step needs it; the previous step's output
                # pack is emitted after gates(0) so the PE never waits on
                # the (long) half-1 quantizer chain at a step boundary.
                pu = ppu.tile([128, 512], F32, tag="pu", name="pu")
                sqv = sbq.tile([128, 512], F32, tag="sqv", name="sqv")
                qf = sbq.tile([128, 512], F32, tag="qf", name="qf")
                scr = (
                    sbs.tile([128, 512], I32, tag="yn", name="yn"),
                    sbs.tile([128, 512], F32, tag="m", name="m"),
                    sbs.tile([128, 512], F32, tag="m2", name="m2"),
                    sbs.tile([128, 512], F32, tag="y2", name="y2"),
                    sbs.tile([128, 512], F32, tag="sm", name="sm"),
                    sbs.tile([128, 512], F32, tag="dct", name="dct"),
                    sbs.tile([128, 512], F32, tag="acc", name="acc"),
                )
                gates(0)
                if pend is not None:
                    pack_out(*pend)
                gates(1)
                wo_mm(0)
                gates(2)
                wo_mm(1)
                clip_half(0, pu, sqv, qf, scr)
                gates(3)
                wo_mm(2)
                wo_mm(3)
                clip_half(1, pu, sqv, qf, scr)
                return (qf, t_idx)

            if unroll <= 0:
                pend = None
                for t in range(T):
                    pend = step(t, pend)
                pack_out(*pend)
            else:
                assert T % unroll == 0
                n_iter = T // unroll
                # PE body exceeds one 256-instruction IRAM block, so arm the
                # branch prefetcher to avoid a ~3-4us ifetch stall per edge.
                with tc.For_i(0, n_iter * unroll, unroll,
                              hint_engines=(mybir.EngineType.PE,)) as iv:
                    pend = None
                    for j in range(unroll):
                        pend = step(iv + j, pend)
                    pack_out(*pend)

            # store recurrent state for segment chaining
            for c in range(CH):
                cs = slice(c * 512, (c + 1) * 512)
                nc.sync.dma_start(
                    out=h1_d[:, :, :].transpose([1, 0, 2])[:, :, cs],
                    in_=h_c[c])
                nc.sync.dma_start(out=x1_d[:, :][:, cs], in_=x_c[c])
            nc.sync.dma_start(out=et1_d[:, :], in_=et)

    nc.finalize()
    return nc


# ---------------- host side ----------------

_module_cache: dict = {}
_exec_cache: dict = {}


def _get_exec(nc):
    """Build (and cache) a jitted SPMD executor for ``nc``.

    Mirrors bass2jax.run_bass_via_pjrt, with two changes: the donated
    output buffers are created on-device by a jitted ``jnp.zeros`` (the
    stock path uploads host zeros through the axon tunnel every call),
    and the jitted callable is cached so repeat calls skip retracing.
    """
    key = id(nc)
    if key in _exec_cache:
        return _exec_cache[key]

    import jax
    import jax.numpy as jnp
    from concourse import bass2jax as b2j

    b2j.install_neuronx_cc_hook()
    partition_name = (
        nc.partition_id_tensor.name if nc.partition_id_tensor else None)

    in_names, out_names, out_avals = [], [], []
    for alloc in nc.m.functions[0].allocations:
        if not isinstance(alloc, mybir.MemoryLocationSet):
            continue
        name = alloc.memorylocations[0].name
        if alloc.kind == "ExternalInput":
            if name != partition_name:
                in_names.append(name)
        elif alloc.kind == "ExternalOutput":
            out_names.append(name)
            out_avals.append(jax.core.ShapedArray(
                tuple(alloc.tensor_shape), mybir.dt.np(alloc.dtype)))
    n_params = len(in_names)
    n_outs = len(out_avals)
    all_names = in_names + out_names
    if partition_name is not None:
        all_names.append(partition_name)
    donate = tuple(range(n_params, n_params + n_outs))

    def _body(*args):
        operands = list(args)
        if partition_name is not None:
            operands.append(b2j.partition_id_tensor())
        outs = b2j._bass_exec_p.bind(
            *operands,
            out_avals=tuple(out_avals),
            in_names=tuple(all_names),
            out_names=tuple(out_names),
            lowering_input_output_aliases=(),
            sim_require_finite=True,
            sim_require_nnan=True,
            nc=nc,
        )
        return tuple(outs)

    devices = jax.devices()[:N_CORES]
    mesh = b2j.Mesh(np.asarray(devices), ("core",))
    in_specs = (b2j.PartitionSpec("core"),) * (n_params + n_outs)
    out_specs = (b2j.PartitionSpec("core"),) * n_outs
    sharded = jax.jit(
        b2j.shard_map(_body, mesh=mesh, in_specs=in_specs,
                      out_specs=out_specs, check_rep=False),
        donate_argnums=donate, keep_unused=True)

    out_sh = jax.sharding.NamedSharding(mesh, b2j.PartitionSpec("core"))
    zeros_fn = jax.jit(
        lambda: tuple(
            jnp.zeros((N_CORES * av.shape[0], *av.shape[1:]), av.dtype)
            for av in out_avals),
        out_shardings=(out_sh,) * n_outs)

    dbg_extra = None
    if nc.dbg_addr is not None:
        dbg_extra = (nc.dbg_addr.name, np.zeros((1, 2), np.uint32))

    state = (sharded, zeros_fn, in_names, out_names, out_avals, dbg_extra)
    _exec_cache[key] = state
    return state


def _stage_inputs(nc, in_maps):
    """Upload per-core inputs to the 8-device mesh once; reusable across
    calls (only the output zeros are donated)."""
    import jax
    sharded, zeros_fn, in_names, out_names, out_avals, dbg_extra = _get_exec(nc)
    if dbg_extra is not None:
        name, z = dbg_extra
        in_maps = [{**m, name: z} for m in in_maps]
    per_core = [[np.asarray(m[name]) for name in in_names] for m in in_maps]
    concat_in = [
        np.concatenate([per_core[c][i] for c in range(N_CORES)], axis=0)
        for i in range(len(in_names))
    ]
    from concourse import bass2jax as b2j
    mesh = b2j.Mesh(np.asarray(jax.devices()[:N_CORES]), ("core",))
    sh = jax.sharding.NamedSharding(mesh, b2j.PartitionSpec("core"))
    staged = [jax.device_put(a, sh) for a in concat_in]
    jax.block_until_ready(staged)
    return staged


NSEG = 4


def _run_staged(nc, staged, nseg=NSEG):
    """Run T as `nseg` chained segments (state stays on device); the
    serial np.asarray fetches of early segments overlap the execution of
    later ones. Returns a list of per-segment global 'out' np arrays."""
    sharded, zeros_fn, in_names, out_names, out_avals, dbg_extra = (
        _get_exec(nc))
    i_h0 = in_names.index("h0")
    i_x0 = in_names.index("x0i")
    i_et = in_names.index("et0")
    o_out = out_names.index("out")
    o_h1 = out_names.index("h1")
    o_x1 = out_names.index("x1")
    o_et = out_names.index("et1")
    cur = list(staged)
    outs = []
    for k in range(nseg):
        zeros = zeros_fn()
        res = sharded(*cur, *zeros)
        outs.append(res[o_out])
        if k + 1 < nseg:
            cur = list(cur)
            cur[i_h0] = res[o_h1]
            cur[i_x0] = res[o_x1]
            cur[i_et] = res[o_et]
    return [np.asarray(o) for o in outs]


def _unpack_global(seg_outs, T, nloc, x0, y0):
    """Segmented uint8-packed int4 deltas -> positions.

    seg_outs: per-segment global arrays [N_CORES*tseg, CH, 512] uint8.
    """
    CH = nloc // 512
    tseg = T // len(seg_outs)
    qs = []
    for p in seg_outs:
        p = p.reshape(N_CORES, tseg, CH, 512)
        q = np.empty((N_CORES, tseg, CH, 512, 2), np.int8)
        q[..., 0] = (p & 15).astype(np.int8) - 8
        q[..., 1] = (p >> 4).astype(np.int8) - 8
        qs.append(q)
    q = np.concatenate(qs, axis=1)  # cores, T, CH, 512, 2
    q = q.transpose(0, 2, 3, 1, 4).reshape(N_CORES * nloc, T, 2)
    pos = np.cumsum(q.astype(np.float32), axis=1, dtype=np.float32)
    pos *= np.float32(S_Q)
    pos[:, :, 0] += np.asarray(x0, np.float32)[:, None]
    pos[:, :, 1] += np.asarray(y0, np.float32)[:, None]
    return pos


def _get_module(T: int, nloc: int, unroll: int):
    key = (T, nloc, unroll)
    if key not in _module_cache:
        _module_cache[key] = build_module(T, nloc, unroll)
    return _module_cache[key]


def _host_prep(inputs, nloc):
    """Build per-core in_maps from full inputs."""
    N = inputs["init_h"].shape[0]
    n_sh = N // N_CORES
    CH = nloc // 512
    W_ih = np.asarray(inputs["W_ih"], np.float32)
    W_hh = np.asarray(inputs["W_hh"], np.float32)
    b_ih = np.asarray(inputs["b_ih"], np.float32)
    b_hh = np.asarray(inputs["b_hh"], np.float32)
    W_out = np.asarray(inputs["W_out"], np.float32)
    b_out = np.asarray(inputs["b_out"], np.float32)

    f16 = np.float16
    wh = np.ascontiguousarray(W_hh.T.reshape(2, 128, 768)).astype(f16)
    wo = np.ascontiguousarray(W_out.T.reshape(2, 128, 2)).astype(f16)

    # K=8 input tails: rows 0-1 = delta cols of W_ih, rows 2-6 = ctx cols,
    # row 7 = bias (the x tile carries a constant 1.0 in row 7).
    wt = np.zeros((8, 6, 128), f16)
    for mt in range(6):
        if mt < 4:
            rows = slice(mt * 128, (mt + 1) * 128)
            bias = (b_ih + b_hh)[rows]
        else:
            rows = slice(512 + (mt - 4) * 128, 512 + (mt - 3) * 128)
            bias = b_ih[rows]
        wt[0:7, mt, :] = W_ih[rows, :].T.astype(f16)
        wt[7, mt, :] = bias.astype(f16)

    # bv cols 6-7 = b_hh n-tile biases (STT scalars); cols 0-5 unused
    bv = np.zeros((128, 8), np.float32)
    for i in range(2):
        bv[:, 6 + i] = b_hh[512 + i * 128:512 + (i + 1) * 128]

    wd2 = np.zeros((128, 128), np.float32)
    for c in range(CH):
        for i in range(2):
            for j in range(2):
                wd2[32 * c + i, 32 * c + j] = 0.5 / (MS * MS)

    wpk = np.zeros((128, 4), np.float32)
    for c in range(CH):
        wpk[32 * c + 0, c] = 1.0
        wpk[32 * c + 1, c] = 16.0

    bpk = np.asarray(b_out, np.float32).reshape(2, 1)

    init_h = np.asarray(inputs["init_h"], np.float32)
    ctx_in = np.asarray(inputs["ctx"], np.float32)

    in_maps = []
    for core in range(N_CORES):
        sl = slice(core * n_sh, (core + 1) * n_sh)
        h0 = np.ascontiguousarray(init_h[sl].T.reshape(2, 128, nloc)).astype(f16)
        x0i = np.zeros((8, nloc), f16)
        x0i[2:7] = ctx_in[sl].T.astype(f16)
        x0i[7] = 1.0
        in_maps.append({
            "h0": h0, "x0i": x0i, "wh": wh,
            "wt": wt, "wo": wo, "wd2": wd2, "wpk": wpk, "bv": bv,
            "bpk": bpk, "et0": np.zeros((128, 512), np.float32),
        })
    return in_maps


def _host_unpack(results, T, nloc, x0, y0):
    """uint8-packed int4 sigma-delta deltas -> pos = pos0 + S_Q*cumsum(q).

    byte = (qx+8) | ((qy+8)<<4), q in [-7, 7].
    """
    CH = nloc // 512
    outs = []
    for r in results:
        p = r["out"]  # uint8 [T, CH, 512]
        q = np.empty((T, CH, 512, 2), np.int8)
        q[..., 0] = (p & 15).astype(np.int8) - 8
        q[..., 1] = (p >> 4).astype(np.int8) - 8
        a = q.transpose(1, 2, 0, 3)  # ch, s, T, 2
        outs.append(a.reshape(nloc, T, 2))
    q = np.concatenate(outs, axis=0).astype(np.float32)  # (N, T, 2)
    pos = np.cumsum(q, axis=1, dtype=np.float32) * np.float32(S_Q)
    pos[:, :, 0] += np.asarray(x0, np.float32)[:, None]
    pos[:, :, 1] += np.asarray(y0, np.float32)[:, None]
    return pos


UNROLL = 2


def kernel(**inputs) -> np.ndarray:
    T = int(inputs["T"])
    N = inputs["init_h"].shape[0]
    nloc = N // N_CORES
    in_maps = _host_prep(inputs, nloc)
    nseg = NSEG if (T % NSEG == 0 and (T // NSEG) % UNROLL == 0) else 1
    try:
        nc = _get_module(T // nseg, nloc, UNROLL)
        seg_outs = _run_staged(nc, _stage_inputs(nc, in_maps), nseg)
        return _unpack_global(seg_outs, T, nloc, inputs["x0"], inputs["y0"])
    except Exception:
        nc = _get_module(T, nloc, UNROLL if T % UNROLL == 0 else 0)
        results = run_bass_kernel_spmd(
            nc, in_maps, core_ids=list(range(N_CORES))).results
        return _host_unpack(results, T, nloc, inputs["x0"], inputs["y0"])

